# revision 59
# baseline (speedup 1.0000x reference)
"""RNN-T joint network kernel for 8 Trainium2 NeuronCores.

Reference computation:
    enc_proj = enc_out @ W_enc.T + b_enc          # [B,T,J]
    pred_proj = pred_out @ W_dec.T + b_dec        # [B,U,J]
    joint    = tanh(enc_proj[:,:,None,:] + dec_proj[:,None,:,:])
    out      = joint @ W_out.T + b_out            # [B,T,U,V]

Shapes (hardcoded): B=4, T=256, U=128, D=512, J=640, V=1024.

Sharding: data-parallel over the B*T = 1024 encoder rows; core k gets
batch b=k//2 and t-range [(k%2)*128, (k%2)*128+128).  Each core computes
its full [128, 128, 1024] output slab.

Algorithm (fp8 residual decomposition):
    tanh(x) = a*x + r(x), a = 0.6225 chosen to minimize Var[r]
    out = (E2[t,:] + D2[u,:] + b_out) + r(x)@Wo
  E2 = a*e@Wo and D2 = a*d@Wo ride tiny f16 GEMMs; the dominant GEMM
  contracts the small-variance residual r in fp8e4 DoubleRow (K=256 per
  pass, full 2x over bf16 at N_out=512).  j-chunks 0-3 (512 of 640) run
  fp8-DR; chunk 4 runs direct-tanh f16.  Everything entering PSUM is
  scaled by S = 2^14 (r*64, W*256); output is written f16 WITHOUT
  descaling (|S*out| < 34k < f16 max) and the host multiplies by 1/S.
  Per-t E2[t,:] is DMA-staged to partition 0 and added into PSUM via a
  K=1 f16 ones-matmul.  Engine layout per t: xp=D+E[t] on GPSIMD (5),
  tanh on ACT (3, pair-merged), rq=(th-xp)*64 -> fp8 on DVE (2 fused
  ln_bwd_dx ops, pair-merged) + 1 combine add, 8 MMs on PE.
"""

import os
import numpy as np

B, T, U, D, J, V = 4, 256, 128, 512, 640, 1024
NCORES = 8
TC = (B * T) // NCORES          # 128 t-rows per core
JC = J // 128                   # 5 j-chunks
DC = D // 128                   # 4 d-chunks
NP = 2                          # fp8 DoubleRow chunk-pairs (j 0..511)
ALPHA = 0.6225                  # linear part of tanh split
SW = 4096.0                     # W_out fp8 scale (|Wo|*4096 < 240 e4m3 max)
S = SW                          # PSUM/output scale (residual stays unscaled)

_CACHE = {}


def _build_bass():
    import concourse.mybir as mybir
    import concourse.tile as tile
    import concourse.bacc as bacc

    f32 = mybir.dt.float32
    f16 = mybir.dt.float16
    f8 = mybir.dt.float8e4

    nc = bacc.Bacc("TRN2", debug=False)

    debug = bool(int(os.environ.get("TRNK_DEBUG", "0")))
    enc_d = nc.dram_tensor("enct", [D, TC], f16, kind="ExternalInput")
    pred_d = nc.dram_tensor("predt", [D, U], f16, kind="ExternalInput")
    wenc_d = nc.dram_tensor("wenct", [D, J], f16, kind="ExternalInput")
    wdec_d = nc.dram_tensor("wdect", [D, J], f16, kind="ExternalInput")
    w8p_d = nc.dram_tensor("w8p", [NP * 128, 2 * V], f8, kind="ExternalInput")
    w4s_d = nc.dram_tensor("w4s", [128, V], f16, kind="ExternalInput")
    wobh_d = nc.dram_tensor("wobh", [512, V], f16, kind="ExternalInput")
    abcomb_d = nc.dram_tensor("abcomb", [128, JC], f32, kind="ExternalInput")
    boutr_d = nc.dram_tensor("boutr", [128, V], f32, kind="ExternalInput")
    ones_d = nc.dram_tensor("onesr", [128, 128], f16, kind="ExternalInput")
    ident_d = nc.dram_tensor("identr", [128, 128], f16, kind="ExternalInput")
    out_d = nc.dram_tensor("out", [TC, U, V], f16, kind="ExternalOutput")
    if debug:
        dbg_e2s = nc.dram_tensor("dbg_e2s", [128, V], f16, kind="ExternalOutput")
        dbg_d2b = nc.dram_tensor("dbg_d2b", [128, V], f32, kind="ExternalOutput")
        dbg_rq0 = nc.dram_tensor("dbg_rq0", [128, 2, U], f8, kind="ExternalOutput")
        dbg_ps0 = nc.dram_tensor("dbg_ps0", [128, V], f32, kind="ExternalOutput")

    enc_ap, pred_ap = enc_d.ap(), pred_d.ap()
    wenc_ap, wdec_ap = wenc_d.ap(), wdec_d.ap()
    out_ap = out_d.ap()

    Tanh = mybir.ActivationFunctionType.Tanh
    Ident = mybir.ActivationFunctionType.Identity
    DR = mybir.MatmulPerfMode.DoubleRow

    with tile.TileContext(nc) as tc:
        with (
            tc.tile_pool(name="consts", bufs=1) as consts,
            tc.tile_pool(name="proj", bufs=1) as proj,
            tc.tile_pool(name="work", bufs=3) as work,
            tc.tile_pool(name="rqp", bufs=3) as rqp,
            tc.tile_pool(name="e2row", bufs=17) as e2rp,
            tc.tile_pool(name="osb", bufs=3) as osbp,
            tc.tile_pool(name="psB", bufs=4, space="PSUM") as psB,
        ):
            # ---- identity first: it feeds the HAM warm-up matmul burst ----
            ident_t = consts.tile([128, 128], f16, tag="identr")
            nc.sync.dma_start(ident_t[:], ident_d.ap()[:])
            wup = psB.tile([128, 128], f32, tag="ps")
            for r in range(30):
                nc.tensor.matmul(wup[:], ident_t[:], ident_t[:],
                                 start=(r == 0), stop=(r == 29))

            # ---- load inputs; projection operands first so PE can start ----
            enc_t, pred_t, wenc_t, wdec_t = [], [], [], []
            for dc in range(DC):
                sl = slice(dc * 128, (dc + 1) * 128)
                a = consts.tile([128, TC], f16, tag=f"enc{dc}")
                nc.sync.dma_start(a[:], enc_ap[sl, :])
                enc_t.append(a)
                p = consts.tile([128, U], f16, tag=f"pred{dc}")
                nc.sync.dma_start(p[:], pred_ap[sl, :])
                pred_t.append(p)
                we = consts.tile([128, J], f16, tag=f"wenc{dc}")
                nc.sync.dma_start(we[:], wenc_ap[sl, :])
                wenc_t.append(we)
                wd = consts.tile([128, J], f16, tag=f"wdec{dc}")
                nc.sync.dma_start(wd[:], wdec_ap[sl, :])
                wdec_t.append(wd)

            abcomb_t = consts.tile([128, JC], f32, tag="abcomb")
            nc.sync.dma_start(abcomb_t[:], abcomb_d.ap()[:])
            w8p_t = []
            for p8 in range(NP):
                w = consts.tile([128, 2, V], f8, tag=f"w8p{p8}")
                nc.sync.dma_start(
                    w[:], w8p_d.ap()[p8 * 128:(p8 + 1) * 128, :])
                w8p_t.append(w)
            w4s_t = consts.tile([128, V], f16, tag="w4s")
            nc.sync.dma_start(w4s_t[:], w4s_d.ap()[:])
            wobh_t = []
            for c in range(4):
                w = consts.tile([128, V], f16, tag=f"wobh{c}")
                nc.sync.dma_start(w[:], wobh_d.ap()[c * 128:(c + 1) * 128, :])
                wobh_t.append(w)
            boutr_t = consts.tile([128, V], f32, tag="boutr")
            nc.sync.dma_start(boutr_t[:], boutr_d.ap()[:])
            ones_t = consts.tile([128, 128], f16, tag="onesr")
            nc.sync.dma_start(ones_t[:], ones_d.ap()[:])

            # ---- projections: E[c][j,t] = a*e, D[c][j,u] = a*(d + bcomb) ----
            E_t, D_t = [], []
            for c in range(JC):
                jsl = slice(c * 128, (c + 1) * 128)
                pse = psB.tile([128, TC], f32, tag="ps")
                for dc in range(DC):
                    nc.tensor.matmul(pse[:], wenc_t[dc][:, jsl], enc_t[dc][:],
                                     start=(dc == 0), stop=(dc == DC - 1))
                e = proj.tile([128, TC], f16, tag=f"E{c}")
                nc.scalar.activation(e[:], pse[:], Ident, bias=0.0, scale=ALPHA)
                E_t.append(e)

                psd = psB.tile([128, U], f32, tag="ps")
                for dc in range(DC):
                    nc.tensor.matmul(psd[:], wdec_t[dc][:, jsl], pred_t[dc][:],
                                     start=(dc == 0), stop=(dc == DC - 1))
                d = proj.tile([128, U], f16, tag=f"D{c}")
                nc.scalar.activation(d[:], psd[:], Ident,
                                     bias=abcomb_t[:, c:c + 1], scale=ALPHA)
                D_t.append(d)

            Add = mybir.AluOpType.add
            G = 8                       # t-rows per elementwise block
            VA = 512                    # ACT-evac width (d2b added via PE)
            BCAST = (128, G, U)

            # ---- per-block elementwise: xp = D+E (DVE, bcast APs),
            #      th = tanh(xp/a) (ACT), rq = th-xp -> fp8 (DVE) ----
            def elementwise_blk(g):
                t0 = g * G
                e2rs = []
                for i in range(G):
                    e2r = e2rp.tile([1, V], f16, tag="e2r")
                    nc.sync.dma_start(e2r[:], e2s[t0 + i:t0 + i + 1, :])
                    e2rs.append(e2r)
                xpG, thG = [], []
                for c in range(JC):
                    xp = work.tile([128, G, U], f16, tag=f"xpG{c}")
                    d_b = D_t[c][:].unsqueeze(1).broadcast_to(BCAST)
                    e_b = (E_t[c][:, t0:t0 + G].unsqueeze(2)
                           .broadcast_to(BCAST))
                    nc.vector.tensor_tensor(xp[:], d_b, e_b, Add)
                    th = work.tile([128, G, U], f16, tag=f"thG{c}")
                    nc.scalar.activation(th[:], xp[:], Tanh, bias=0.0,
                                         scale=1.0 / ALPHA)
                    xpG.append(xp)
                    thG.append(th)
                rqP = []
                for p8 in range(NP):
                    r = rqp.tile([128, 2, G * U], f8, tag=f"rq{p8}")
                    for i in range(2):
                        c = p8 * 2 + i
                        nc.vector.affine_then_add(
                            r[:, i, :],
                            xpG[c][:].rearrange("p g u -> p (g u)"),
                            thG[c][:].rearrange("p g u -> p (g u)"),
                            -1.0, 0.0)
                    rqP.append(r)
                return xpG, thG, rqP, e2rs

            def matmuls(t, i, rqP, thG, e2rs):
                usl = slice(i * U, (i + 1) * U)
                ps = psB.tile([128, V], f32, tag="ps")
                for v in range(2):
                    vsl = slice(v * 512, (v + 1) * 512)
                    for p8 in range(NP):
                        nc.tensor.matmul(ps[:, vsl], rqP[p8][:, :, usl],
                                         w8p_t[p8][:, :, vsl],
                                         start=(p8 == 0), stop=False,
                                         perf_mode=DR)
                e2r = e2rs[i]
                for v in range(2):
                    vsl = slice(v * 512, (v + 1) * 512)
                    nc.tensor.matmul(ps[:, vsl], thG[4][:, i, :],
                                     w4s_t[:, vsl], start=False, stop=False)
                    if v == 0:
                        nc.tensor.matmul(ps[:, vsl], ones_t[0:1, :],
                                         e2r[0:1, vsl],
                                         start=False, stop=False)
                        nc.tensor.matmul(ps[:, vsl], ident_t[:],
                                         d2b16[:, vsl],
                                         start=False, stop=True)
                    else:
                        nc.tensor.matmul(ps[:, vsl], ones_t[0:1, :],
                                         e2r[0:1, vsl],
                                         start=False, stop=True)
                return ps

            def combine(t, ps):
                osb = osbp.tile([128, V], f16, tag="osb")
                nc.scalar.activation(osb[:, 0:VA], ps[:, 0:VA], Ident,
                                     bias=0.0, scale=1.0)
                nc.vector.tensor_add(osb[:, VA:V], ps[:, VA:V],
                                     d2b16[:, VA:V])
                nc.sync.dma_start(out_ap[t], osb[:])

            # ---- mini-GEMMs: e2s[t,v] = S*E2, d2bS[u,v] = S*(D2 + b_out) ----
            pse2 = psB.tile([128, V], f32, tag="ps")
            for v in range(2):
                vsl = slice(v * 512, (v + 1) * 512)
                for c in range(4):
                    nc.tensor.matmul(pse2[:, vsl], E_t[c][:], wobh_t[c][:, vsl],
                                     start=(c == 0), stop=(c == 3))
            e2s = proj.tile([128, V], f16, tag="e2s")
            nc.scalar.activation(e2s[:], pse2[:], Ident, bias=0.0, scale=1.0)

            psd2 = psB.tile([128, V], f32, tag="ps")
            for v in range(2):
                vsl = slice(v * 512, (v + 1) * 512)
                for c in range(4):
                    nc.tensor.matmul(psd2[:, vsl], D_t[c][:], wobh_t[c][:, vsl],
                                     start=(c == 0), stop=(c == 3))
            d2bf = proj.tile([128, V], f32, tag="d2bf")
            nc.vector.tensor_add(d2bf[:], psd2[:], boutr_t[:])
            d2b16 = proj.tile([128, V], f16, tag="d2b16")
            nc.vector.tensor_copy(d2b16[:], d2bf[:])

            if debug:
                nc.sync.dma_start(dbg_e2s.ap()[:], e2s[:])
                nc.sync.dma_start(dbg_d2b.ap()[:], d2bf[:])

            # ---- main loop, software-pipelined by G-block ----
            NB = TC // G
            pend = {0: elementwise_blk(0)}
            prev = None     # (t, ps)
            for g in range(NB):
                xpG, thG, rqP, e2rs = pend.pop(g)
                if g + 1 < NB:
                    pend[g + 1] = elementwise_blk(g + 1)
                for i in range(G):
                    t = g * G + i
                    ps = matmuls(t, i, rqP, thG, e2rs)
                    if prev is not None:
                        combine(prev[0], prev[1])
                    if debug and t == 0:
                        nc.sync.dma_start(dbg_rq0.ap()[:], rqP[0][:, :, 0:U])
                        pscp = osbp.tile([128, V], f32, tag="pscp")
                        nc.vector.tensor_copy(pscp[:], ps[:])
                        nc.sync.dma_start(dbg_ps0.ap()[:], pscp[:])
                    prev = (t, ps)
            combine(prev[0], prev[1])

    nc.compile()
    return nc


def _host_prep(enc_out, pred_out, W_enc, b_enc, W_dec, b_dec, W_out, b_out):
    import concourse.mybir as mybir
    f8_np = np.dtype(mybir.dt.np(mybir.dt.float8e4))
    f16_np = np.float16

    wencT = np.ascontiguousarray(np.asarray(W_enc, np.float32).T).astype(f16_np)
    wdecT = np.ascontiguousarray(np.asarray(W_dec, np.float32).T).astype(f16_np)
    woT = np.ascontiguousarray(np.asarray(W_out, np.float32).T)  # [J, V]
    # fp8 DoubleRow pair layout: w8p[p8*128+p, i*V+v] = Wo[p8*256+i*128+p, v]*SW
    w8p = np.ascontiguousarray(
        (woT[:512] * SW).reshape(NP, 2, 128, V).transpose(0, 2, 1, 3)
        .reshape(NP * 128, 2 * V)).astype(f8_np)
    w4s = np.ascontiguousarray(woT[512:] * S).astype(f16_np)
    wobh = np.ascontiguousarray(woT[:512] * S).astype(f16_np)
    abcomb = np.ascontiguousarray(
        (ALPHA * (np.asarray(b_enc, np.float32) + np.asarray(b_dec, np.float32)))
        .reshape(JC, 128).T)
    boutr = np.ascontiguousarray(
        np.broadcast_to(np.asarray(b_out, np.float32) * S, (128, V)))
    onesr = np.ones((128, 128), f16_np)
    identr = np.eye(128, dtype=f16_np)

    in_maps = []
    for k in range(NCORES):
        b, th = k // 2, (k % 2) * TC
        encT = np.ascontiguousarray(
            np.asarray(enc_out[b, th:th + TC], np.float32).T).astype(f16_np)
        predT = np.ascontiguousarray(
            np.asarray(pred_out[b], np.float32).T).astype(f16_np)
        in_maps.append({
            "enct": encT, "predt": predT, "wenct": wencT, "wdect": wdecT,
            "w8p": w8p, "w4s": w4s, "wobh": wobh, "abcomb": abcomb,
            "boutr": boutr, "onesr": onesr, "identr": identr,
        })
    return in_maps


def kernel(enc_out, pred_out, W_enc, b_enc, W_dec, b_dec, W_out, b_out):
    from concourse import bass_utils

    if "nc" not in _CACHE:
        _CACHE["nc"] = _build_bass()
    nc = _CACHE["nc"]

    in_maps = _host_prep(enc_out, pred_out, W_enc, b_enc, W_dec, b_dec,
                         W_out, b_out)

    trace = bool(int(os.environ.get("TRNK_PROFILE", "0")))
    res = bass_utils.run_bass_kernel_spmd(
        nc, in_maps, core_ids=list(range(NCORES)), trace=trace)
    kernel.last_exec_ns = res.exec_time_ns

    full = np.empty((B, T, U, V), np.float32)
    for k in range(NCORES):
        b, th = k // 2, (k % 2) * TC
        full[b, th:th + TC] = res.results[k]["out"].astype(np.float32)
    full *= 1.0 / S
    return full


kernel.last_exec_ns = None


# revision 60
# speedup vs baseline: 1.0485x; 1.0485x over previous
"""RNN-T joint network kernel for 8 Trainium2 NeuronCores.

Reference computation:
    enc_proj = enc_out @ W_enc.T + b_enc          # [B,T,J]
    pred_proj = pred_out @ W_dec.T + b_dec        # [B,U,J]
    joint    = tanh(enc_proj[:,:,None,:] + dec_proj[:,None,:,:])
    out      = joint @ W_out.T + b_out            # [B,T,U,V]

Shapes (hardcoded): B=4, T=256, U=128, D=512, J=640, V=1024.

Sharding: data-parallel over the B*T = 1024 encoder rows; core k gets
batch b=k//2 and t-range [(k%2)*128, (k%2)*128+128).  Each core computes
its full [128, 128, 1024] output slab.

Algorithm (fp8 residual decomposition):
    tanh(x) = a*x + r(x), a = 0.6225 chosen to minimize Var[r]
    out = (E2[t,:] + D2[u,:] + b_out) + r(x)@Wo
  E2 = a*e@Wo and D2 = a*d@Wo ride tiny f16 GEMMs; the dominant GEMM
  contracts the small-variance residual r in fp8e4 DoubleRow (K=256 per
  pass, full 2x over bf16 at N_out=512).  j-chunks 0-3 (512 of 640) run
  fp8-DR; chunk 4 runs direct-tanh f16.  Everything entering PSUM is
  scaled by S = 2^14 (r*64, W*256); output is written f16 WITHOUT
  descaling (|S*out| < 34k < f16 max) and the host multiplies by 1/S.
  Per-t E2[t,:] is DMA-staged to partition 0 and added into PSUM via a
  K=1 f16 ones-matmul.  Engine layout per t: xp=D+E[t] on GPSIMD (5),
  tanh on ACT (3, pair-merged), rq=(th-xp)*64 -> fp8 on DVE (2 fused
  ln_bwd_dx ops, pair-merged) + 1 combine add, 8 MMs on PE.
"""

import os
import numpy as np

B, T, U, D, J, V = 4, 256, 128, 512, 640, 1024
NCORES = 8
TC = (B * T) // NCORES          # 128 t-rows per core
JC = J // 128                   # 5 j-chunks
DC = D // 128                   # 4 d-chunks
NP = 2                          # fp8 DoubleRow chunk-pairs (j 0..511)
ALPHA = 0.6225                  # linear part of tanh split
SW = 4096.0                     # W_out fp8 scale (|Wo|*4096 < 240 e4m3 max)
S = SW                          # PSUM/output scale (residual stays unscaled)

_CACHE = {}


def _build_bass():
    import concourse.mybir as mybir
    import concourse.tile as tile
    import concourse.bacc as bacc

    f32 = mybir.dt.float32
    f16 = mybir.dt.float16
    f8 = mybir.dt.float8e4

    nc = bacc.Bacc("TRN2", debug=False)

    debug = bool(int(os.environ.get("TRNK_DEBUG", "0")))
    enc_d = nc.dram_tensor("enct", [D, TC], f16, kind="ExternalInput")
    pred_d = nc.dram_tensor("predt", [D, U], f16, kind="ExternalInput")
    wenc_d = nc.dram_tensor("wenct", [D, J], f16, kind="ExternalInput")
    wdec_d = nc.dram_tensor("wdect", [D, J], f16, kind="ExternalInput")
    w8p_d = nc.dram_tensor("w8p", [NP * 128, 2 * V], f8, kind="ExternalInput")
    w4s_d = nc.dram_tensor("w4s", [128, V], f16, kind="ExternalInput")
    wobh_d = nc.dram_tensor("wobh", [512, V], f16, kind="ExternalInput")
    abcomb_d = nc.dram_tensor("abcomb", [128, JC], f32, kind="ExternalInput")
    boutr_d = nc.dram_tensor("boutr", [128, V], f32, kind="ExternalInput")
    ones_d = nc.dram_tensor("onesr", [128, 128], f16, kind="ExternalInput")
    ident_d = nc.dram_tensor("identr", [128, 128], f16, kind="ExternalInput")
    out_d = nc.dram_tensor("out", [TC, U, V], f16, kind="ExternalOutput")
    if debug:
        dbg_e2s = nc.dram_tensor("dbg_e2s", [128, V], f16, kind="ExternalOutput")
        dbg_d2b = nc.dram_tensor("dbg_d2b", [128, V], f32, kind="ExternalOutput")
        dbg_rq0 = nc.dram_tensor("dbg_rq0", [128, 2, U], f8, kind="ExternalOutput")
        dbg_ps0 = nc.dram_tensor("dbg_ps0", [128, V], f32, kind="ExternalOutput")

    enc_ap, pred_ap = enc_d.ap(), pred_d.ap()
    wenc_ap, wdec_ap = wenc_d.ap(), wdec_d.ap()
    out_ap = out_d.ap()

    Tanh = mybir.ActivationFunctionType.Tanh
    Ident = mybir.ActivationFunctionType.Identity
    DR = mybir.MatmulPerfMode.DoubleRow

    with tile.TileContext(nc) as tc:
        with (
            tc.tile_pool(name="consts", bufs=1) as consts,
            tc.tile_pool(name="proj", bufs=1) as proj,
            tc.tile_pool(name="work", bufs=3) as work,
            tc.tile_pool(name="rqp", bufs=3) as rqp,
            tc.tile_pool(name="e2row", bufs=9) as e2rp,
            tc.tile_pool(name="osb", bufs=3) as osbp,
            tc.tile_pool(name="psB", bufs=4, space="PSUM") as psB,
        ):
            # ---- identity first: it feeds the HAM warm-up matmul burst ----
            ident_t = consts.tile([128, 128], f16, tag="identr")
            nc.sync.dma_start(ident_t[:], ident_d.ap()[:])
            wup = psB.tile([128, 128], f32, tag="ps")
            for r in range(30):
                nc.tensor.matmul(wup[:], ident_t[:], ident_t[:],
                                 start=(r == 0), stop=(r == 29))

            # ---- load inputs; projection operands first so PE can start ----
            enc_t, pred_t, wenc_t, wdec_t = [], [], [], []
            for dc in range(DC):
                sl = slice(dc * 128, (dc + 1) * 128)
                a = consts.tile([128, TC], f16, tag=f"enc{dc}")
                nc.sync.dma_start(a[:], enc_ap[sl, :])
                enc_t.append(a)
                p = consts.tile([128, U], f16, tag=f"pred{dc}")
                nc.sync.dma_start(p[:], pred_ap[sl, :])
                pred_t.append(p)
                we = consts.tile([128, J], f16, tag=f"wenc{dc}")
                nc.sync.dma_start(we[:], wenc_ap[sl, :])
                wenc_t.append(we)
                wd = consts.tile([128, J], f16, tag=f"wdec{dc}")
                nc.sync.dma_start(wd[:], wdec_ap[sl, :])
                wdec_t.append(wd)

            abcomb_t = consts.tile([128, JC], f32, tag="abcomb")
            nc.sync.dma_start(abcomb_t[:], abcomb_d.ap()[:])
            w8p_t = []
            for p8 in range(NP):
                w = consts.tile([128, 2, V], f8, tag=f"w8p{p8}")
                nc.sync.dma_start(
                    w[:], w8p_d.ap()[p8 * 128:(p8 + 1) * 128, :])
                w8p_t.append(w)
            w4s_t = consts.tile([128, V], f16, tag="w4s")
            nc.sync.dma_start(w4s_t[:], w4s_d.ap()[:])
            wobh_t = []
            for c in range(4):
                w = consts.tile([128, V], f16, tag=f"wobh{c}")
                nc.sync.dma_start(w[:], wobh_d.ap()[c * 128:(c + 1) * 128, :])
                wobh_t.append(w)
            boutr_t = consts.tile([128, V], f32, tag="boutr")
            nc.sync.dma_start(boutr_t[:], boutr_d.ap()[:])
            ones_t = consts.tile([128, 128], f16, tag="onesr")
            nc.sync.dma_start(ones_t[:], ones_d.ap()[:])

            # ---- projections: E[c][j,t] = a*e, D[c][j,u] = a*(d + bcomb) ----
            E_t, D_t = [], []
            for c in range(JC):
                jsl = slice(c * 128, (c + 1) * 128)
                pse = psB.tile([128, TC], f32, tag="ps")
                for dc in range(DC):
                    nc.tensor.matmul(pse[:], wenc_t[dc][:, jsl], enc_t[dc][:],
                                     start=(dc == 0), stop=(dc == DC - 1))
                e = proj.tile([128, TC], f16, tag=f"E{c}")
                nc.scalar.activation(e[:], pse[:], Ident, bias=0.0, scale=ALPHA)
                E_t.append(e)

                psd = psB.tile([128, U], f32, tag="ps")
                for dc in range(DC):
                    nc.tensor.matmul(psd[:], wdec_t[dc][:, jsl], pred_t[dc][:],
                                     start=(dc == 0), stop=(dc == DC - 1))
                d = proj.tile([128, U], f16, tag=f"D{c}")
                nc.scalar.activation(d[:], psd[:], Ident,
                                     bias=abcomb_t[:, c:c + 1], scale=ALPHA)
                D_t.append(d)

            Add = mybir.AluOpType.add
            G = 4                       # t-rows per elementwise block
            VA = 512                    # ACT-evac width (d2b added via PE)
            BCAST = (128, G, U)

            # ---- per-block elementwise: xp = D+E (DVE, bcast APs),
            #      th = tanh(xp/a) (ACT), rq = th-xp -> fp8 (DVE) ----
            def elementwise_blk(g):
                t0 = g * G
                e2rs = []
                for i in range(G):
                    e2r = e2rp.tile([1, V], f16, tag="e2r")
                    nc.sync.dma_start(e2r[:], e2s[t0 + i:t0 + i + 1, :])
                    e2rs.append(e2r)
                xpG, thG = [], []
                for c in range(JC):
                    xp = work.tile([128, G, U], f16, tag=f"xpG{c}")
                    d_b = D_t[c][:].unsqueeze(1).broadcast_to(BCAST)
                    e_b = (E_t[c][:, t0:t0 + G].unsqueeze(2)
                           .broadcast_to(BCAST))
                    nc.vector.tensor_tensor(xp[:], d_b, e_b, Add)
                    th = work.tile([128, G, U], f16, tag=f"thG{c}")
                    nc.scalar.activation(th[:], xp[:], Tanh, bias=0.0,
                                         scale=1.0 / ALPHA)
                    xpG.append(xp)
                    thG.append(th)
                rqP = []
                for p8 in range(NP):
                    r = rqp.tile([128, 2, G * U], f8, tag=f"rq{p8}")
                    for i in range(2):
                        c = p8 * 2 + i
                        nc.vector.affine_then_add(
                            r[:, i, :],
                            xpG[c][:].rearrange("p g u -> p (g u)"),
                            thG[c][:].rearrange("p g u -> p (g u)"),
                            -1.0, 0.0)
                    rqP.append(r)
                return xpG, thG, rqP, e2rs

            def matmuls(t, i, rqP, thG, e2rs):
                usl = slice(i * U, (i + 1) * U)
                ps = psB.tile([128, V], f32, tag="ps")
                for v in range(2):
                    vsl = slice(v * 512, (v + 1) * 512)
                    for p8 in range(NP):
                        nc.tensor.matmul(ps[:, vsl], rqP[p8][:, :, usl],
                                         w8p_t[p8][:, :, vsl],
                                         start=(p8 == 0), stop=False,
                                         perf_mode=DR)
                e2r = e2rs[i]
                for v in range(2):
                    vsl = slice(v * 512, (v + 1) * 512)
                    nc.tensor.matmul(ps[:, vsl], thG[4][:, i, :],
                                     w4s_t[:, vsl], start=False, stop=False)
                    if v == 0:
                        nc.tensor.matmul(ps[:, vsl], ones_t[0:1, :],
                                         e2r[0:1, vsl],
                                         start=False, stop=False)
                        nc.tensor.matmul(ps[:, vsl], ident_t[:],
                                         d2b16[:, vsl],
                                         start=False, stop=True)
                    else:
                        nc.tensor.matmul(ps[:, vsl], ones_t[0:1, :],
                                         e2r[0:1, vsl],
                                         start=False, stop=True)
                return ps

            def combine(t, ps):
                osb = osbp.tile([128, V], f16, tag="osb")
                nc.scalar.activation(osb[:, 0:VA], ps[:, 0:VA], Ident,
                                     bias=0.0, scale=1.0)
                nc.vector.tensor_add(osb[:, VA:V], ps[:, VA:V],
                                     d2b16[:, VA:V])
                nc.sync.dma_start(out_ap[t], osb[:])

            # ---- mini-GEMMs: e2s[t,v] = S*E2, d2bS[u,v] = S*(D2 + b_out) ----
            pse2 = psB.tile([128, V], f32, tag="ps")
            for v in range(2):
                vsl = slice(v * 512, (v + 1) * 512)
                for c in range(4):
                    nc.tensor.matmul(pse2[:, vsl], E_t[c][:], wobh_t[c][:, vsl],
                                     start=(c == 0), stop=(c == 3))
            e2s = proj.tile([128, V], f16, tag="e2s")
            nc.scalar.activation(e2s[:], pse2[:], Ident, bias=0.0, scale=1.0)

            psd2 = psB.tile([128, V], f32, tag="ps")
            for v in range(2):
                vsl = slice(v * 512, (v + 1) * 512)
                for c in range(4):
                    nc.tensor.matmul(psd2[:, vsl], D_t[c][:], wobh_t[c][:, vsl],
                                     start=(c == 0), stop=(c == 3))
            d2bf = proj.tile([128, V], f32, tag="d2bf")
            nc.vector.tensor_add(d2bf[:], psd2[:], boutr_t[:])
            d2b16 = proj.tile([128, V], f16, tag="d2b16")
            nc.vector.tensor_copy(d2b16[:], d2bf[:])

            if debug:
                nc.sync.dma_start(dbg_e2s.ap()[:], e2s[:])
                nc.sync.dma_start(dbg_d2b.ap()[:], d2bf[:])

            # ---- main loop, software-pipelined by G-block ----
            NB = TC // G
            pend = {0: elementwise_blk(0)}
            prev = None     # (t, ps)
            for g in range(NB):
                xpG, thG, rqP, e2rs = pend.pop(g)
                if g + 1 < NB:
                    pend[g + 1] = elementwise_blk(g + 1)
                for i in range(G):
                    t = g * G + i
                    ps = matmuls(t, i, rqP, thG, e2rs)
                    if prev is not None:
                        combine(prev[0], prev[1])
                    if debug and t == 0:
                        nc.sync.dma_start(dbg_rq0.ap()[:], rqP[0][:, :, 0:U])
                        pscp = osbp.tile([128, V], f32, tag="pscp")
                        nc.vector.tensor_copy(pscp[:], ps[:])
                        nc.sync.dma_start(dbg_ps0.ap()[:], pscp[:])
                    prev = (t, ps)
            combine(prev[0], prev[1])

    nc.compile()
    return nc


def _host_prep(enc_out, pred_out, W_enc, b_enc, W_dec, b_dec, W_out, b_out):
    import concourse.mybir as mybir
    f8_np = np.dtype(mybir.dt.np(mybir.dt.float8e4))
    f16_np = np.float16

    wencT = np.ascontiguousarray(np.asarray(W_enc, np.float32).T).astype(f16_np)
    wdecT = np.ascontiguousarray(np.asarray(W_dec, np.float32).T).astype(f16_np)
    woT = np.ascontiguousarray(np.asarray(W_out, np.float32).T)  # [J, V]
    # fp8 DoubleRow pair layout: w8p[p8*128+p, i*V+v] = Wo[p8*256+i*128+p, v]*SW
    w8p = np.ascontiguousarray(
        (woT[:512] * SW).reshape(NP, 2, 128, V).transpose(0, 2, 1, 3)
        .reshape(NP * 128, 2 * V)).astype(f8_np)
    w4s = np.ascontiguousarray(woT[512:] * S).astype(f16_np)
    wobh = np.ascontiguousarray(woT[:512] * S).astype(f16_np)
    abcomb = np.ascontiguousarray(
        (ALPHA * (np.asarray(b_enc, np.float32) + np.asarray(b_dec, np.float32)))
        .reshape(JC, 128).T)
    boutr = np.ascontiguousarray(
        np.broadcast_to(np.asarray(b_out, np.float32) * S, (128, V)))
    onesr = np.ones((128, 128), f16_np)
    identr = np.eye(128, dtype=f16_np)

    in_maps = []
    for k in range(NCORES):
        b, th = k // 2, (k % 2) * TC
        encT = np.ascontiguousarray(
            np.asarray(enc_out[b, th:th + TC], np.float32).T).astype(f16_np)
        predT = np.ascontiguousarray(
            np.asarray(pred_out[b], np.float32).T).astype(f16_np)
        in_maps.append({
            "enct": encT, "predt": predT, "wenct": wencT, "wdect": wdecT,
            "w8p": w8p, "w4s": w4s, "wobh": wobh, "abcomb": abcomb,
            "boutr": boutr, "onesr": onesr, "identr": identr,
        })
    return in_maps


def kernel(enc_out, pred_out, W_enc, b_enc, W_dec, b_dec, W_out, b_out):
    from concourse import bass_utils

    if "nc" not in _CACHE:
        _CACHE["nc"] = _build_bass()
    nc = _CACHE["nc"]

    in_maps = _host_prep(enc_out, pred_out, W_enc, b_enc, W_dec, b_dec,
                         W_out, b_out)

    trace = bool(int(os.environ.get("TRNK_PROFILE", "0")))
    res = bass_utils.run_bass_kernel_spmd(
        nc, in_maps, core_ids=list(range(NCORES)), trace=trace)
    kernel.last_exec_ns = res.exec_time_ns

    full = np.empty((B, T, U, V), np.float32)
    for k in range(NCORES):
        b, th = k // 2, (k % 2) * TC
        full[b, th:th + TC] = res.results[k]["out"].astype(np.float32)
    full *= 1.0 / S
    return full


kernel.last_exec_ns = None


# revision 61
# speedup vs baseline: 1.1476x; 1.0945x over previous
"""RNN-T joint network kernel for 8 Trainium2 NeuronCores.

Reference computation:
    enc_proj = enc_out @ W_enc.T + b_enc          # [B,T,J]
    pred_proj = pred_out @ W_dec.T + b_dec        # [B,U,J]
    joint    = tanh(enc_proj[:,:,None,:] + dec_proj[:,None,:,:])
    out      = joint @ W_out.T + b_out            # [B,T,U,V]

Shapes (hardcoded): B=4, T=256, U=128, D=512, J=640, V=1024.

Sharding: data-parallel over the B*T = 1024 encoder rows; core k gets
batch b=k//2 and t-range [(k%2)*128, (k%2)*128+128).  Each core computes
its full [128, 128, 1024] output slab.

Algorithm (fp8 residual decomposition):
    tanh(x) = a*x + r(x), a = 0.6225 chosen to minimize Var[r]
    out = (E2[t,:] + D2[u,:] + b_out) + r(x)@Wo
  E2 = a*e@Wo and D2 = a*d@Wo ride tiny f16 GEMMs; the dominant GEMM
  contracts the small-variance residual r in fp8e4 DoubleRow (K=256 per
  pass, full 2x over bf16 at N_out=512).  j-chunks 0-3 (512 of 640) run
  fp8-DR; chunk 4 runs direct-tanh f16.  Everything entering PSUM is
  scaled by S = 2^14 (r*64, W*256); output is written f16 WITHOUT
  descaling (|S*out| < 34k < f16 max) and the host multiplies by 1/S.
  Per-t E2[t,:] is DMA-staged to partition 0 and added into PSUM via a
  K=1 f16 ones-matmul.  Engine layout per t: xp=D+E[t] on GPSIMD (5),
  tanh on ACT (3, pair-merged), rq=(th-xp)*64 -> fp8 on DVE (2 fused
  ln_bwd_dx ops, pair-merged) + 1 combine add, 8 MMs on PE.
"""

import os
import numpy as np

B, T, U, D, J, V = 4, 256, 128, 512, 640, 1024
NCORES = 8
TC = (B * T) // NCORES          # 128 t-rows per core
JC = J // 128                   # 5 j-chunks
DC = D // 128                   # 4 d-chunks
NP = 2                          # fp8 DoubleRow chunk-pairs (j 0..511)
ALPHA = 0.6225                  # linear part of tanh split
SW = 4096.0                     # W_out fp8 scale (|Wo|*4096 < 240 e4m3 max)
S = SW                          # PSUM/output scale (residual stays unscaled)

_CACHE = {}


def _build_bass():
    import concourse.mybir as mybir
    import concourse.tile as tile
    import concourse.bacc as bacc

    f32 = mybir.dt.float32
    f16 = mybir.dt.float16
    f8 = mybir.dt.float8e4

    nc = bacc.Bacc("TRN2", debug=False)

    debug = bool(int(os.environ.get("TRNK_DEBUG", "0")))
    enc_d = nc.dram_tensor("enct", [D, TC], f16, kind="ExternalInput")
    pred_d = nc.dram_tensor("predt", [D, U], f16, kind="ExternalInput")
    wenc_d = nc.dram_tensor("wenct", [D, J], f16, kind="ExternalInput")
    wdec_d = nc.dram_tensor("wdect", [D, J], f16, kind="ExternalInput")
    w8p_d = nc.dram_tensor("w8p", [NP * 128, 2 * V], f8, kind="ExternalInput")
    w4s_d = nc.dram_tensor("w4s", [128, V], f16, kind="ExternalInput")
    wobh_d = nc.dram_tensor("wobh", [512, V], f16, kind="ExternalInput")
    abcomb_d = nc.dram_tensor("abcomb", [128, JC], f32, kind="ExternalInput")
    boutr_d = nc.dram_tensor("boutr", [128, V], f32, kind="ExternalInput")
    ones_d = nc.dram_tensor("onesr", [128, 128], f16, kind="ExternalInput")
    ident_d = nc.dram_tensor("identr", [128, 128], f16, kind="ExternalInput")
    out_d = nc.dram_tensor("out", [TC, U, V], f16, kind="ExternalOutput")
    if debug:
        dbg_e2s = nc.dram_tensor("dbg_e2s", [128, V], f16, kind="ExternalOutput")
        dbg_d2b = nc.dram_tensor("dbg_d2b", [128, V], f32, kind="ExternalOutput")
        dbg_rq0 = nc.dram_tensor("dbg_rq0", [128, 2, U], f8, kind="ExternalOutput")
        dbg_ps0 = nc.dram_tensor("dbg_ps0", [128, V], f32, kind="ExternalOutput")

    enc_ap, pred_ap = enc_d.ap(), pred_d.ap()
    wenc_ap, wdec_ap = wenc_d.ap(), wdec_d.ap()
    out_ap = out_d.ap()

    Tanh = mybir.ActivationFunctionType.Tanh
    Ident = mybir.ActivationFunctionType.Identity
    DR = mybir.MatmulPerfMode.DoubleRow

    with tile.TileContext(nc) as tc:
        with (
            tc.tile_pool(name="consts", bufs=1) as consts,
            tc.tile_pool(name="proj", bufs=1) as proj,
            tc.tile_pool(name="work", bufs=3) as work,
            tc.tile_pool(name="rqp", bufs=3) as rqp,
            tc.tile_pool(name="e2row", bufs=9) as e2rp,
            tc.tile_pool(name="osb", bufs=3) as osbp,
            tc.tile_pool(name="psB", bufs=4, space="PSUM") as psB,
        ):
            # ---- identity first: it feeds the HAM warm-up matmul burst ----
            ident_t = consts.tile([128, 128], f16, tag="identr")
            nc.sync.dma_start(ident_t[:], ident_d.ap()[:])
            wup = psB.tile([128, 128], f32, tag="ps")
            for r in range(30):
                nc.tensor.matmul(wup[:], ident_t[:], ident_t[:],
                                 start=(r == 0), stop=(r == 29))

            # ---- load inputs; projection operands first so PE can start ----
            enc_t, pred_t, wenc_t, wdec_t = [], [], [], []
            for dc in range(DC):
                sl = slice(dc * 128, (dc + 1) * 128)
                a = consts.tile([128, TC], f16, tag=f"enc{dc}")
                nc.sync.dma_start(a[:], enc_ap[sl, :])
                enc_t.append(a)
                p = consts.tile([128, U], f16, tag=f"pred{dc}")
                nc.sync.dma_start(p[:], pred_ap[sl, :])
                pred_t.append(p)
                we = consts.tile([128, J], f16, tag=f"wenc{dc}")
                nc.sync.dma_start(we[:], wenc_ap[sl, :])
                wenc_t.append(we)
                wd = consts.tile([128, J], f16, tag=f"wdec{dc}")
                nc.sync.dma_start(wd[:], wdec_ap[sl, :])
                wdec_t.append(wd)

            abcomb_t = consts.tile([128, JC], f32, tag="abcomb")
            nc.sync.dma_start(abcomb_t[:], abcomb_d.ap()[:])
            w8p_t = []
            for p8 in range(NP):
                w = consts.tile([128, 2, V], f8, tag=f"w8p{p8}")
                nc.sync.dma_start(
                    w[:], w8p_d.ap()[p8 * 128:(p8 + 1) * 128, :])
                w8p_t.append(w)
            w4s_t = consts.tile([128, V], f16, tag="w4s")
            nc.sync.dma_start(w4s_t[:], w4s_d.ap()[:])
            wobh_t = []
            for c in range(4):
                w = consts.tile([128, V], f16, tag=f"wobh{c}")
                nc.sync.dma_start(w[:], wobh_d.ap()[c * 128:(c + 1) * 128, :])
                wobh_t.append(w)
            boutr_t = consts.tile([128, V], f32, tag="boutr")
            nc.sync.dma_start(boutr_t[:], boutr_d.ap()[:])
            ones_t = consts.tile([128, 128], f16, tag="onesr")
            nc.sync.dma_start(ones_t[:], ones_d.ap()[:])

            # ---- projections: E[c][j,t] = a*e, D[c][j,u] = a*(d + bcomb) ----
            E_t, D_t = [], []
            for c in range(JC):
                jsl = slice(c * 128, (c + 1) * 128)
                pse = psB.tile([128, TC], f32, tag="ps")
                for dc in range(DC):
                    nc.tensor.matmul(pse[:], wenc_t[dc][:, jsl], enc_t[dc][:],
                                     start=(dc == 0), stop=(dc == DC - 1))
                e = proj.tile([128, TC], f16, tag=f"E{c}")
                nc.scalar.activation(e[:], pse[:], Ident, bias=0.0, scale=ALPHA)
                E_t.append(e)

                psd = psB.tile([128, U], f32, tag="ps")
                for dc in range(DC):
                    nc.tensor.matmul(psd[:], wdec_t[dc][:, jsl], pred_t[dc][:],
                                     start=(dc == 0), stop=(dc == DC - 1))
                d = proj.tile([128, U], f16, tag=f"D{c}")
                nc.scalar.activation(d[:], psd[:], Ident,
                                     bias=abcomb_t[:, c:c + 1], scale=ALPHA)
                D_t.append(d)

            Add = mybir.AluOpType.add
            G = 4                       # t-rows per elementwise block
            VA = 512                    # ACT-evac width (d2b added via PE)
            BCAST = (128, G, U)

            # ---- per-block elementwise: xp = D+E (DVE, bcast APs),
            #      th = tanh(xp/a) (ACT), rq = th-xp -> fp8 (DVE) ----
            def elementwise_blk(g):
                t0 = g * G
                e2rs = []
                for i in range(G):
                    e2r = e2rp.tile([1, V], f16, tag="e2r")
                    nc.sync.dma_start(e2r[:], e2s[t0 + i:t0 + i + 1, :])
                    e2rs.append(e2r)
                xpG, thG = [], []
                for c in range(JC):
                    xp = work.tile([128, G, U], f16, tag=f"xpG{c}")
                    d_b = D_t[c][:].unsqueeze(1).broadcast_to(BCAST)
                    e_b = (E_t[c][:, t0:t0 + G].unsqueeze(2)
                           .broadcast_to(BCAST))
                    nc.vector.tensor_tensor(xp[:], d_b, e_b, Add)
                    th = work.tile([128, G, U], f16, tag=f"thG{c}")
                    nc.scalar.activation(th[:], xp[:], Tanh, bias=0.0,
                                         scale=1.0 / ALPHA)
                    xpG.append(xp)
                    thG.append(th)
                rqP = []
                for p8 in range(NP):
                    r = rqp.tile([128, 2, G * U], f8, tag=f"rq{p8}")
                    for i in range(2):
                        c = p8 * 2 + i
                        nc.vector.affine_then_add(
                            r[:, i, :],
                            xpG[c][:].rearrange("p g u -> p (g u)"),
                            thG[c][:].rearrange("p g u -> p (g u)"),
                            -1.0, 0.0)
                    rqP.append(r)
                return xpG, thG, rqP, e2rs

            def matmuls(t, i, rqP, thG, e2rs):
                usl = slice(i * U, (i + 1) * U)
                ps = psB.tile([128, V], f32, tag="ps")
                # stationary-major order: consecutive MMs share their lhsT so
                # redundant LDWEIGHTS can be skipped/hidden.
                e2r = e2rs[i]
                for p8 in range(NP):
                    for v in range(2):
                        vsl = slice(v * 512, (v + 1) * 512)
                        nc.tensor.matmul(ps[:, vsl], rqP[p8][:, :, usl],
                                         w8p_t[p8][:, :, vsl],
                                         start=(p8 == 0), stop=False,
                                         perf_mode=DR)
                for v in range(2):
                    vsl = slice(v * 512, (v + 1) * 512)
                    nc.tensor.matmul(ps[:, vsl], thG[4][:, i, :],
                                     w4s_t[:, vsl], start=False, stop=False)
                for v in range(2):
                    vsl = slice(v * 512, (v + 1) * 512)
                    nc.tensor.matmul(ps[:, vsl], ones_t[0:1, :],
                                     e2r[0:1, vsl],
                                     start=False, stop=(v == 1))
                nc.tensor.matmul(ps[:, 0:512], ident_t[:], d2b16[:, 0:512],
                                 start=False, stop=True)
                return ps

            def combine(t, ps):
                osb = osbp.tile([128, V], f16, tag="osb")
                nc.scalar.activation(osb[:, 0:VA], ps[:, 0:VA], Ident,
                                     bias=0.0, scale=1.0)
                nc.vector.tensor_add(osb[:, VA:V], ps[:, VA:V],
                                     d2b16[:, VA:V])
                nc.sync.dma_start(out_ap[t], osb[:])

            # ---- mini-GEMMs: e2s[t,v] = S*E2, d2bS[u,v] = S*(D2 + b_out) ----
            pse2 = psB.tile([128, V], f32, tag="ps")
            for v in range(2):
                vsl = slice(v * 512, (v + 1) * 512)
                for c in range(4):
                    nc.tensor.matmul(pse2[:, vsl], E_t[c][:], wobh_t[c][:, vsl],
                                     start=(c == 0), stop=(c == 3))
            e2s = proj.tile([128, V], f16, tag="e2s")
            nc.scalar.activation(e2s[:], pse2[:], Ident, bias=0.0, scale=1.0)

            psd2 = psB.tile([128, V], f32, tag="ps")
            for v in range(2):
                vsl = slice(v * 512, (v + 1) * 512)
                for c in range(4):
                    nc.tensor.matmul(psd2[:, vsl], D_t[c][:], wobh_t[c][:, vsl],
                                     start=(c == 0), stop=(c == 3))
            d2bf = proj.tile([128, V], f32, tag="d2bf")
            nc.vector.tensor_add(d2bf[:], psd2[:], boutr_t[:])
            d2b16 = proj.tile([128, V], f16, tag="d2b16")
            nc.vector.tensor_copy(d2b16[:], d2bf[:])

            if debug:
                nc.sync.dma_start(dbg_e2s.ap()[:], e2s[:])
                nc.sync.dma_start(dbg_d2b.ap()[:], d2bf[:])

            # ---- main loop, software-pipelined by G-block ----
            NB = TC // G
            pend = {0: elementwise_blk(0)}
            prev = None     # (t, ps)
            for g in range(NB):
                xpG, thG, rqP, e2rs = pend.pop(g)
                if g + 1 < NB:
                    pend[g + 1] = elementwise_blk(g + 1)
                for i in range(G):
                    t = g * G + i
                    ps = matmuls(t, i, rqP, thG, e2rs)
                    if prev is not None:
                        combine(prev[0], prev[1])
                    if debug and t == 0:
                        nc.sync.dma_start(dbg_rq0.ap()[:], rqP[0][:, :, 0:U])
                        pscp = osbp.tile([128, V], f32, tag="pscp")
                        nc.vector.tensor_copy(pscp[:], ps[:])
                        nc.sync.dma_start(dbg_ps0.ap()[:], pscp[:])
                    prev = (t, ps)
            combine(prev[0], prev[1])

    nc.compile()
    return nc


def _host_prep(enc_out, pred_out, W_enc, b_enc, W_dec, b_dec, W_out, b_out):
    import concourse.mybir as mybir
    f8_np = np.dtype(mybir.dt.np(mybir.dt.float8e4))
    f16_np = np.float16

    wencT = np.ascontiguousarray(np.asarray(W_enc, np.float32).T).astype(f16_np)
    wdecT = np.ascontiguousarray(np.asarray(W_dec, np.float32).T).astype(f16_np)
    woT = np.ascontiguousarray(np.asarray(W_out, np.float32).T)  # [J, V]
    # fp8 DoubleRow pair layout: w8p[p8*128+p, i*V+v] = Wo[p8*256+i*128+p, v]*SW
    w8p = np.ascontiguousarray(
        (woT[:512] * SW).reshape(NP, 2, 128, V).transpose(0, 2, 1, 3)
        .reshape(NP * 128, 2 * V)).astype(f8_np)
    w4s = np.ascontiguousarray(woT[512:] * S).astype(f16_np)
    wobh = np.ascontiguousarray(woT[:512] * S).astype(f16_np)
    abcomb = np.ascontiguousarray(
        (ALPHA * (np.asarray(b_enc, np.float32) + np.asarray(b_dec, np.float32)))
        .reshape(JC, 128).T)
    boutr = np.ascontiguousarray(
        np.broadcast_to(np.asarray(b_out, np.float32) * S, (128, V)))
    onesr = np.ones((128, 128), f16_np)
    identr = np.eye(128, dtype=f16_np)

    in_maps = []
    for k in range(NCORES):
        b, th = k // 2, (k % 2) * TC
        encT = np.ascontiguousarray(
            np.asarray(enc_out[b, th:th + TC], np.float32).T).astype(f16_np)
        predT = np.ascontiguousarray(
            np.asarray(pred_out[b], np.float32).T).astype(f16_np)
        in_maps.append({
            "enct": encT, "predt": predT, "wenct": wencT, "wdect": wdecT,
            "w8p": w8p, "w4s": w4s, "wobh": wobh, "abcomb": abcomb,
            "boutr": boutr, "onesr": onesr, "identr": identr,
        })
    return in_maps


def kernel(enc_out, pred_out, W_enc, b_enc, W_dec, b_dec, W_out, b_out):
    from concourse import bass_utils

    if "nc" not in _CACHE:
        _CACHE["nc"] = _build_bass()
    nc = _CACHE["nc"]

    in_maps = _host_prep(enc_out, pred_out, W_enc, b_enc, W_dec, b_dec,
                         W_out, b_out)

    trace = bool(int(os.environ.get("TRNK_PROFILE", "0")))
    res = bass_utils.run_bass_kernel_spmd(
        nc, in_maps, core_ids=list(range(NCORES)), trace=trace)
    kernel.last_exec_ns = res.exec_time_ns

    full = np.empty((B, T, U, V), np.float32)
    for k in range(NCORES):
        b, th = k // 2, (k % 2) * TC
        full[b, th:th + TC] = res.results[k]["out"].astype(np.float32)
    full *= 1.0 / S
    return full


kernel.last_exec_ns = None


# revision 63
# speedup vs baseline: 1.1524x; 1.0043x over previous
"""RNN-T joint network kernel for 8 Trainium2 NeuronCores.

Reference computation:
    enc_proj = enc_out @ W_enc.T + b_enc          # [B,T,J]
    pred_proj = pred_out @ W_dec.T + b_dec        # [B,U,J]
    joint    = tanh(enc_proj[:,:,None,:] + dec_proj[:,None,:,:])
    out      = joint @ W_out.T + b_out            # [B,T,U,V]

Shapes (hardcoded): B=4, T=256, U=128, D=512, J=640, V=1024.

Sharding: data-parallel over the B*T = 1024 encoder rows; core k gets
batch b=k//2 and t-range [(k%2)*128, (k%2)*128+128).  Each core computes
its full [128, 128, 1024] output slab.

Algorithm (fp8 residual decomposition):
    tanh(x) = a*x + r(x), a = 0.6225 chosen to minimize Var[r]
    out = (E2[t,:] + D2[u,:] + b_out) + r(x)@Wo
  E2 = a*e@Wo and D2 = a*d@Wo ride tiny f16 GEMMs; the dominant GEMM
  contracts the small-variance residual r in fp8e4 DoubleRow (K=256 per
  pass, full 2x over bf16 at N_out=512).  j-chunks 0-3 (512 of 640) run
  fp8-DR; chunk 4 runs direct-tanh f16.  Everything entering PSUM is
  scaled by S = 2^14 (r*64, W*256); output is written f16 WITHOUT
  descaling (|S*out| < 34k < f16 max) and the host multiplies by 1/S.
  Per-t E2[t,:] is DMA-staged to partition 0 and added into PSUM via a
  K=1 f16 ones-matmul.  Engine layout per t: xp=D+E[t] on GPSIMD (5),
  tanh on ACT (3, pair-merged), rq=(th-xp)*64 -> fp8 on DVE (2 fused
  ln_bwd_dx ops, pair-merged) + 1 combine add, 8 MMs on PE.
"""

import os
import numpy as np

B, T, U, D, J, V = 4, 256, 128, 512, 640, 1024
NCORES = 8
TC = (B * T) // NCORES          # 128 t-rows per core
JC = J // 128                   # 5 j-chunks
DC = D // 128                   # 4 d-chunks
NP = 2                          # fp8 DoubleRow chunk-pairs (j 0..511)
ALPHA = 0.6225                  # linear part of tanh split
SW = 4096.0                     # W_out fp8 scale (|Wo|*4096 < 240 e4m3 max)
S = SW                          # PSUM/output scale (residual stays unscaled)

_CACHE = {}


def _build_bass():
    import concourse.mybir as mybir
    import concourse.tile as tile
    import concourse.bacc as bacc

    f32 = mybir.dt.float32
    f16 = mybir.dt.float16
    f8 = mybir.dt.float8e4

    nc = bacc.Bacc("TRN2", debug=False)

    debug = bool(int(os.environ.get("TRNK_DEBUG", "0")))
    enc_d = nc.dram_tensor("enct", [D, TC], f16, kind="ExternalInput")
    pred_d = nc.dram_tensor("predt", [D, U], f16, kind="ExternalInput")
    wenc_d = nc.dram_tensor("wenct", [D, J], f16, kind="ExternalInput")
    wdec_d = nc.dram_tensor("wdect", [D, J], f16, kind="ExternalInput")
    w8p_d = nc.dram_tensor("w8p", [NP * 128, 2 * V], f8, kind="ExternalInput")
    w4s_d = nc.dram_tensor("w4s", [128, V], f16, kind="ExternalInput")
    wobh_d = nc.dram_tensor("wobh", [512, V], f16, kind="ExternalInput")
    abcomb_d = nc.dram_tensor("abcomb", [128, JC], f32, kind="ExternalInput")
    boutr_d = nc.dram_tensor("boutr", [128, V], f32, kind="ExternalInput")
    ones_d = nc.dram_tensor("onesr", [128, 128], f16, kind="ExternalInput")
    ident_d = nc.dram_tensor("identr", [128, 128], f16, kind="ExternalInput")
    out_d = nc.dram_tensor("out", [TC, U, V], f16, kind="ExternalOutput")
    if debug:
        dbg_e2s = nc.dram_tensor("dbg_e2s", [128, V], f16, kind="ExternalOutput")
        dbg_d2b = nc.dram_tensor("dbg_d2b", [128, V], f32, kind="ExternalOutput")
        dbg_rq0 = nc.dram_tensor("dbg_rq0", [128, 2, U], f8, kind="ExternalOutput")
        dbg_ps0 = nc.dram_tensor("dbg_ps0", [128, V], f32, kind="ExternalOutput")

    enc_ap, pred_ap = enc_d.ap(), pred_d.ap()
    wenc_ap, wdec_ap = wenc_d.ap(), wdec_d.ap()
    out_ap = out_d.ap()

    Tanh = mybir.ActivationFunctionType.Tanh
    Ident = mybir.ActivationFunctionType.Identity
    DR = mybir.MatmulPerfMode.DoubleRow

    with tile.TileContext(nc) as tc:
        with (
            tc.tile_pool(name="consts", bufs=1) as consts,
            tc.tile_pool(name="proj", bufs=1) as proj,
            tc.tile_pool(name="work", bufs=3) as work,
            tc.tile_pool(name="rqp", bufs=3) as rqp,
            tc.tile_pool(name="e2row", bufs=9) as e2rp,
            tc.tile_pool(name="osb", bufs=3) as osbp,
            tc.tile_pool(name="psB", bufs=4, space="PSUM") as psB,
        ):
            # ---- identity first: it feeds the HAM warm-up matmul burst ----
            ident_t = consts.tile([128, 128], f16, tag="identr")
            nc.sync.dma_start(ident_t[:], ident_d.ap()[:])
            wup = psB.tile([128, 128], f32, tag="ps")
            for r in range(80):
                nc.tensor.matmul(wup[:], ident_t[:], ident_t[:],
                                 start=(r == 0), stop=(r == 79))

            # ---- load inputs; projection operands first so PE can start ----
            enc_t, pred_t, wenc_t, wdec_t = [], [], [], []
            for dc in range(DC):
                sl = slice(dc * 128, (dc + 1) * 128)
                a = consts.tile([128, TC], f16, tag=f"enc{dc}")
                nc.sync.dma_start(a[:], enc_ap[sl, :])
                enc_t.append(a)
                p = consts.tile([128, U], f16, tag=f"pred{dc}")
                nc.sync.dma_start(p[:], pred_ap[sl, :])
                pred_t.append(p)
                we = consts.tile([128, J], f16, tag=f"wenc{dc}")
                nc.sync.dma_start(we[:], wenc_ap[sl, :])
                wenc_t.append(we)
                wd = consts.tile([128, J], f16, tag=f"wdec{dc}")
                nc.sync.dma_start(wd[:], wdec_ap[sl, :])
                wdec_t.append(wd)

            abcomb_t = consts.tile([128, JC], f32, tag="abcomb")
            nc.sync.dma_start(abcomb_t[:], abcomb_d.ap()[:])
            w8p_t = []
            for p8 in range(NP):
                w = consts.tile([128, 2, V], f8, tag=f"w8p{p8}")
                nc.sync.dma_start(
                    w[:], w8p_d.ap()[p8 * 128:(p8 + 1) * 128, :])
                w8p_t.append(w)
            w4s_t = consts.tile([128, V], f16, tag="w4s")
            nc.sync.dma_start(w4s_t[:], w4s_d.ap()[:])
            wobh_t = []
            for c in range(4):
                w = consts.tile([128, V], f16, tag=f"wobh{c}")
                nc.sync.dma_start(w[:], wobh_d.ap()[c * 128:(c + 1) * 128, :])
                wobh_t.append(w)
            boutr_t = consts.tile([128, V], f32, tag="boutr")
            nc.sync.dma_start(boutr_t[:], boutr_d.ap()[:])
            ones_t = consts.tile([128, 128], f16, tag="onesr")
            nc.sync.dma_start(ones_t[:], ones_d.ap()[:])

            # ---- projections: E[c][j,t] = a*e, D[c][j,u] = a*(d + bcomb) ----
            E_t, D_t = [], []
            for c in range(JC):
                jsl = slice(c * 128, (c + 1) * 128)
                pse = psB.tile([128, TC], f32, tag="ps")
                for dc in range(DC):
                    nc.tensor.matmul(pse[:], wenc_t[dc][:, jsl], enc_t[dc][:],
                                     start=(dc == 0), stop=(dc == DC - 1))
                e = proj.tile([128, TC], f16, tag=f"E{c}")
                nc.scalar.activation(e[:], pse[:], Ident, bias=0.0, scale=ALPHA)
                E_t.append(e)

                psd = psB.tile([128, U], f32, tag="ps")
                for dc in range(DC):
                    nc.tensor.matmul(psd[:], wdec_t[dc][:, jsl], pred_t[dc][:],
                                     start=(dc == 0), stop=(dc == DC - 1))
                d = proj.tile([128, U], f16, tag=f"D{c}")
                nc.scalar.activation(d[:], psd[:], Ident,
                                     bias=abcomb_t[:, c:c + 1], scale=ALPHA)
                D_t.append(d)

            Add = mybir.AluOpType.add
            G = 4                       # t-rows per elementwise block
            VA = 512                    # ACT-evac width (d2b added via PE)
            BCAST = (128, G, U)

            # ---- per-block elementwise: xp = D+E (DVE, bcast APs),
            #      th = tanh(xp/a) (ACT), rq = th-xp -> fp8 (DVE) ----
            def elementwise_blk(g):
                t0 = g * G
                e2rs = []
                for i in range(G):
                    e2r = e2rp.tile([1, V], f16, tag="e2r")
                    nc.sync.dma_start(e2r[:], e2s[t0 + i:t0 + i + 1, :])
                    e2rs.append(e2r)
                xpG, thG = [], []
                for c in range(JC):
                    xp = work.tile([128, G, U], f16, tag=f"xpG{c}")
                    d_b = D_t[c][:].unsqueeze(1).broadcast_to(BCAST)
                    e_b = (E_t[c][:, t0:t0 + G].unsqueeze(2)
                           .broadcast_to(BCAST))
                    nc.vector.tensor_tensor(xp[:], d_b, e_b, Add)
                    th = work.tile([128, G, U], f16, tag=f"thG{c}")
                    nc.scalar.activation(th[:], xp[:], Tanh, bias=0.0,
                                         scale=1.0 / ALPHA)
                    xpG.append(xp)
                    thG.append(th)
                rqP = []
                for p8 in range(NP):
                    r = rqp.tile([128, 2, G * U], f8, tag=f"rq{p8}")
                    for i in range(2):
                        c = p8 * 2 + i
                        nc.vector.affine_then_add(
                            r[:, i, :],
                            xpG[c][:].rearrange("p g u -> p (g u)"),
                            thG[c][:].rearrange("p g u -> p (g u)"),
                            -1.0, 0.0)
                    rqP.append(r)
                return xpG, thG, rqP, e2rs

            def matmuls(t, i, rqP, thG, e2rs):
                # stationary-major order, paired over (i, i+1): consecutive
                # MMs share their lhsT so redundant LDWEIGHTS are skipped.
                ps_a = psB.tile([128, V], f32, tag="ps")
                ps_b = psB.tile([128, V], f32, tag="ps")
                pss = [ps_a, ps_b]
                for p8 in range(NP):
                    for k in range(2):
                        u2 = slice((i + k) * U, (i + k + 1) * U)
                        for v in range(2):
                            vsl = slice(v * 512, (v + 1) * 512)
                            nc.tensor.matmul(pss[k][:, vsl],
                                             rqP[p8][:, :, u2],
                                             w8p_t[p8][:, :, vsl],
                                             start=(p8 == 0), stop=False,
                                             perf_mode=DR)
                for k in range(2):
                    for v in range(2):
                        vsl = slice(v * 512, (v + 1) * 512)
                        nc.tensor.matmul(pss[k][:, vsl], thG[4][:, i + k, :],
                                         w4s_t[:, vsl],
                                         start=False, stop=False)
                for k in range(2):
                    for v in range(2):
                        vsl = slice(v * 512, (v + 1) * 512)
                        nc.tensor.matmul(pss[k][:, vsl], ones_t[0:1, :],
                                         e2rs[i + k][0:1, vsl],
                                         start=False, stop=(v == 1))
                for k in range(2):
                    nc.tensor.matmul(pss[k][:, 0:512], ident_t[:],
                                     d2b16[:, 0:512],
                                     start=False, stop=True)
                return pss

            def combine(t, ps):
                osb = osbp.tile([128, V], f16, tag="osb")
                nc.scalar.activation(osb[:, 0:VA], ps[:, 0:VA], Ident,
                                     bias=0.0, scale=1.0)
                nc.vector.tensor_add(osb[:, VA:V], ps[:, VA:V],
                                     d2b16[:, VA:V])
                nc.sync.dma_start(out_ap[t], osb[:])

            # ---- mini-GEMMs: e2s[t,v] = S*E2, d2bS[u,v] = S*(D2 + b_out) ----
            pse2 = psB.tile([128, V], f32, tag="ps")
            for v in range(2):
                vsl = slice(v * 512, (v + 1) * 512)
                for c in range(4):
                    nc.tensor.matmul(pse2[:, vsl], E_t[c][:], wobh_t[c][:, vsl],
                                     start=(c == 0), stop=(c == 3))
            e2s = proj.tile([128, V], f16, tag="e2s")
            nc.scalar.activation(e2s[:], pse2[:], Ident, bias=0.0, scale=1.0)

            psd2 = psB.tile([128, V], f32, tag="ps")
            for v in range(2):
                vsl = slice(v * 512, (v + 1) * 512)
                for c in range(4):
                    nc.tensor.matmul(psd2[:, vsl], D_t[c][:], wobh_t[c][:, vsl],
                                     start=(c == 0), stop=(c == 3))
            d2bf = proj.tile([128, V], f32, tag="d2bf")
            nc.vector.tensor_add(d2bf[:], psd2[:], boutr_t[:])
            d2b16 = proj.tile([128, V], f16, tag="d2b16")
            nc.vector.tensor_copy(d2b16[:], d2bf[:])

            if debug:
                nc.sync.dma_start(dbg_e2s.ap()[:], e2s[:])
                nc.sync.dma_start(dbg_d2b.ap()[:], d2bf[:])

            # ---- main loop, software-pipelined by G-block ----
            NB = TC // G
            pend = {0: elementwise_blk(0)}
            prev = None     # (t, ps)
            for g in range(NB):
                xpG, thG, rqP, e2rs = pend.pop(g)
                if g + 1 < NB:
                    pend[g + 1] = elementwise_blk(g + 1)
                for i in range(0, G, 2):
                    t = g * G + i
                    pss = matmuls(t, i, rqP, thG, e2rs)
                    for k in range(2):
                        if prev is not None:
                            combine(prev[0], prev[1])
                        prev = (t + k, pss[k])
            combine(prev[0], prev[1])

    nc.compile()
    return nc


def _host_prep(enc_out, pred_out, W_enc, b_enc, W_dec, b_dec, W_out, b_out):
    import concourse.mybir as mybir
    f8_np = np.dtype(mybir.dt.np(mybir.dt.float8e4))
    f16_np = np.float16

    wencT = np.ascontiguousarray(np.asarray(W_enc, np.float32).T).astype(f16_np)
    wdecT = np.ascontiguousarray(np.asarray(W_dec, np.float32).T).astype(f16_np)
    woT = np.ascontiguousarray(np.asarray(W_out, np.float32).T)  # [J, V]
    # fp8 DoubleRow pair layout: w8p[p8*128+p, i*V+v] = Wo[p8*256+i*128+p, v]*SW
    w8p = np.ascontiguousarray(
        (woT[:512] * SW).reshape(NP, 2, 128, V).transpose(0, 2, 1, 3)
        .reshape(NP * 128, 2 * V)).astype(f8_np)
    w4s = np.ascontiguousarray(woT[512:] * S).astype(f16_np)
    wobh = np.ascontiguousarray(woT[:512] * S).astype(f16_np)
    abcomb = np.ascontiguousarray(
        (ALPHA * (np.asarray(b_enc, np.float32) + np.asarray(b_dec, np.float32)))
        .reshape(JC, 128).T)
    boutr = np.ascontiguousarray(
        np.broadcast_to(np.asarray(b_out, np.float32) * S, (128, V)))
    onesr = np.ones((128, 128), f16_np)
    identr = np.eye(128, dtype=f16_np)

    in_maps = []
    for k in range(NCORES):
        b, th = k // 2, (k % 2) * TC
        encT = np.ascontiguousarray(
            np.asarray(enc_out[b, th:th + TC], np.float32).T).astype(f16_np)
        predT = np.ascontiguousarray(
            np.asarray(pred_out[b], np.float32).T).astype(f16_np)
        in_maps.append({
            "enct": encT, "predt": predT, "wenct": wencT, "wdect": wdecT,
            "w8p": w8p, "w4s": w4s, "wobh": wobh, "abcomb": abcomb,
            "boutr": boutr, "onesr": onesr, "identr": identr,
        })
    return in_maps


def kernel(enc_out, pred_out, W_enc, b_enc, W_dec, b_dec, W_out, b_out):
    from concourse import bass_utils

    if "nc" not in _CACHE:
        _CACHE["nc"] = _build_bass()
    nc = _CACHE["nc"]

    in_maps = _host_prep(enc_out, pred_out, W_enc, b_enc, W_dec, b_dec,
                         W_out, b_out)

    trace = bool(int(os.environ.get("TRNK_PROFILE", "0")))
    res = bass_utils.run_bass_kernel_spmd(
        nc, in_maps, core_ids=list(range(NCORES)), trace=trace)
    kernel.last_exec_ns = res.exec_time_ns

    full = np.empty((B, T, U, V), np.float32)
    for k in range(NCORES):
        b, th = k // 2, (k % 2) * TC
        full[b, th:th + TC] = res.results[k]["out"].astype(np.float32)
    full *= 1.0 / S
    return full


kernel.last_exec_ns = None
